# revision 25
# baseline (speedup 1.0000x reference)
"""Trainium2 Bass kernel for nn_FSMNSeleNetV3 (FSMN stack + channel maxpool + decoder).

Self-contained: hardcodes all shapes from the problem spec and only imports
numpy + the concourse stack from /opt/trn_rl_repo.

Sharding: pure data parallel over batch. Each of the 8 cores processes 4
batches x 4 channels = 16 independent sequences of T=2048 tokens.

Layout: activations use an even/odd time-split layout, all in bf16 (fp32
PSUM accumulation). The 128-dim expand stream e is [128 feat, T] with
columns 0:1024 = even times, 1024:2048 = odd times. The 64-dim FSMN h
stream uses an interleaved pair layout (partition 2c = channel c even
times, 2c+1 = odd times); the conv output o uses a blocked pair layout
(partitions 0:63 = even, 64:127 = odd) so the expand can run as K=64
row-tiled concurrent matmul pairs (duplicated weight halves at PE rows 0
and 64 share the rhs stream).

FSMN conv: in pair layout the 11-tap depthwise conv collapses to 7
pair-shift matmuls per 512-column window. Each pass is a full 128x128 bf16
matmul whose weight is a banded matrix of per-channel 2x2 blocks with
interleaved input rows and blocked output columns (built on the host); all
taps accumulate in fp32 PSUM. The conv identity term (o = h + left +
right) is folded into the k=0 tap weights; the layer residual is folded
into the PSUM evacuation as a DVE tensor_tensor add.

The interleaved h layout keeps every matmul destination at PSUM base
partition 0 (the ISA rejects dst base 64): shrink even/odd are two M=128
matmuls with zero-interleaved weight columns accumulating into the same
bank.

Scheduling: sequences are emitted in software-pipelined pairs (stage-
interleaved) so the PE always has an independent matmul stream to fill
cross-engine latency gaps (keeps the PE HAM clock-gate warm). Matmuls that
share a stationary operand are emitted back-to-back; PSUM evacuations are
single wide-FD instructions. bf16 weights enable fast weight load (FWL).
"""

import sys

sys.path.insert(0, "/opt/trn_rl_repo")
from contextlib import ExitStack

import numpy as np

import concourse.bass as bass  # noqa: F401
import concourse.mybir as mybir
import concourse.tile as tile
from concourse import bacc
from concourse.bass_utils import run_bass_kernel_spmd

F32 = mybir.dt.float32
F32R = mybir.dt.float32r
BF16 = mybir.dt.bfloat16
AF = mybir.ActivationFunctionType
OP = mybir.AluOpType

NCORES = 8
B, T, C, F = 32, 2048, 4, 120
DL, DP, L, LO, RO, S = 128, 64, 5, 10, 1, 5
BPC = B // NCORES  # batches per core
SEQ = BPC * C  # sequences per core
NP_ = T // 2  # pair columns per sequence (1024)
HALO = 5  # left pair halo (k down to -5)
HW_ = HALO + NP_ + 1  # h buffer width: 1030
NK = 7  # pair-shift passes, k = kk - 5 in [-5 .. +1]
NH = 6  # static h buffers

# packed bf16 weight tensor column offsets
OFF_WE0 = 0
OFF_WEDUP = OFF_WE0 + DL
OFF_WSIL = OFF_WEDUP + L * DL
OFF_WCONV = OFF_WSIL + L * 2 * DL
OFF_WD = OFF_WCONV + L * NK * 2 * DP
WPK_COLS = OFF_WD + 8


def build_nc():
    nc = bacc.Bacc("TRN2", target_bir_lowering=False, debug=False, num_devices=NCORES)

    xt_d = nc.dram_tensor("xt", [SEQ, F, T], BF16, kind="ExternalInput")
    we0_d = nc.dram_tensor("we0", [F, DL], BF16, kind="ExternalInput")
    wpk_d = nc.dram_tensor("wpk", [DL, WPK_COLS], BF16, kind="ExternalInput")
    wpk32_d = nc.dram_tensor("wpk32", [DL, 8], F32, kind="ExternalInput")
    out_d = nc.dram_tensor("out", [BPC, S, T], F32, kind="ExternalOutput")

    with tile.TileContext(nc) as tc, ExitStack() as ctx:
        wp = ctx.enter_context(tc.tile_pool(name="weights", bufs=1))
        xp = ctx.enter_context(tc.tile_pool(name="x", bufs=5))
        ep = ctx.enter_context(tc.tile_pool(name="e", bufs=5))
        op_ = ctx.enter_context(tc.tile_pool(name="o", bufs=6))
        fp = ctx.enter_context(tc.tile_pool(name="f", bufs=6))
        pp = ctx.enter_context(tc.tile_pool(name="pooled", bufs=2))
        osb = ctx.enter_context(tc.tile_pool(name="osb", bufs=2))
        eps = ctx.enter_context(tc.tile_pool(name="eps", bufs=4, space="PSUM"))
        hps = ctx.enter_context(tc.tile_pool(name="hps", bufs=2, space="PSUM"))
        cps = ctx.enter_context(tc.tile_pool(name="cps", bufs=2, space="PSUM"))

        # --- weights / constants: tiny early DMAs gate unit-0; the big
        # packed DMA is deferred behind the first x loads ---
        wpk32_sb = wp.tile([DL, 8], F32)
        nc.sync.dma_start(out=wpk32_sb[:], in_=wpk32_d[:])
        we0_tile = wp.tile([F, DL], BF16)
        nc.sync.dma_start(out=we0_tile[:], in_=we0_d[:])
        wpk_sb = wp.tile([DL, WPK_COLS], BF16)

        def load_wpk():
            # expand+shrink weights (cols OFF_WEDUP..OFF_WCONV), then conv
            # weights per layer, then the decoder tail
            nc.sync.dma_start(
                out=wpk_sb[:, OFF_WEDUP:OFF_WCONV], in_=wpk_d[:, OFF_WEDUP:OFF_WCONV]
            )
            for l in range(L):
                c0 = OFF_WCONV + l * NK * 2 * DP
                c1 = OFF_WCONV + (l + 1) * NK * 2 * DP
                nc.sync.dma_start(out=wpk_sb[:, c0:c1], in_=wpk_d[:, c0:c1])
            nc.sync.dma_start(
                out=wpk_sb[:, OFF_WD:WPK_COLS], in_=wpk_d[:, OFF_WD:WPK_COLS]
            )

        we0_sb = we0_tile[:]

        def wedup_at(l, q):
            c = OFF_WEDUP + l * DL
            return wpk_sb[q : q + DP, c : c + DL]

        def wsil_at(l, half):
            c = OFF_WSIL + (l * 2 + half) * DL
            return wpk_sb[:, c : c + DL]

        def wconv_at(l, kk):
            c = OFF_WCONV + (l * NK + kk) * 2 * DP
            return wpk_sb[:, c : c + 2 * DP]

        wd_sb = wpk_sb[:, OFF_WD : OFF_WD + S]
        bias_sb = wpk32_sb
        bd_sb = wpk32_sb[0:S, 6:7]

        # PE warmup: garbage matmuls with no data deps run immediately at
        # kernel start, keeping the HAM clock-gate warm through the initial
        # DMA wait (results are never read)
        warm = eps.tile([DL, 512], F32, tag="pe", name="warm")
        for _ in range(6):
            nc.tensor.matmul(warm[:], wpk_sb[:, 0:DL], wpk_sb[:, 0:512])

        # static h buffers: halo columns zeroed once, data region rewritten
        # per (seq, layer) via the shrink evacuation
        h_tiles = []
        for i in range(NH):
            t = wp.tile([2 * DP, HW_], BF16, tag=f"h{i}", name=f"h{i}")
            nc.gpsimd.memset(t[:, 0:HALO], 0.0)
            nc.gpsimd.memset(t[:, HALO + NP_ : HW_], 0.0)
            h_tiles.append(t)

        class Seq:
            def __init__(self, seq):
                self.seq = seq
                self.e = None
                self.o = None
                self.f = None

        hctr = [0]

        def stage_load(st, chunks=1):
            st.x = xp.tile([F, T], BF16, name="x_sb")
            step = T // chunks
            for i in range(chunks):
                nc.sync.dma_start(
                    out=st.x[:, i * step : (i + 1) * step],
                    in_=xt_d[st.seq][:, i * step : (i + 1) * step],
                )

        def stage_unit0(st):
            st.e = ep.tile([DL, T], BF16, name="e_sb")
            for w in range(4):
                pe = eps.tile([DL, 512], F32, tag="pe", name="pe")
                nc.tensor.matmul(pe[:], we0_sb, st.x[:, w * 512 : (w + 1) * 512])
                nc.scalar.activation(
                    st.e[:, w * 512 : (w + 1) * 512],
                    pe[:],
                    AF.Relu,
                    bias=bias_sb[:, 0:1],
                    scale=1.0,
                )

        def expand(dst_sb, lcol, bias_col, o_prev):
            # o_prev blocked: rows 0:63 = even half, 64:127 = odd half.
            # K=64 row-tiled pairs (weights duplicated at rows 0 and 64)
            # stream concurrently and share the rhs columns.
            for w in range(2):
                ws_ = slice(w * 512, (w + 1) * 512)
                pes = []
                for half in range(2):
                    q = half * DP
                    pe = eps.tile([DL, 512], F32, tag="pe", name="pe")
                    nc.tensor.matmul(
                        pe[:],
                        wedup_at(lcol, q),
                        o_prev[q : q + DP, ws_],
                        tile_position=(q, 0),
                    )
                    pes.append(pe)
                for half in range(2):
                    nc.scalar.activation(
                        dst_sb[:, half * NP_ + w * 512 : half * NP_ + (w + 1) * 512],
                        pes[half][:],
                        AF.Relu,
                        bias=bias_sb[:, bias_col : bias_col + 1],
                        scale=1.0,
                    )

        def stage_layer(st, l):
            if l > 0:
                e_new = ep.tile([DL, T], BF16, name="e_sb")
                expand(e_new, l - 1, l, st.o)
                st.e = e_new

            # ---- shrink l: weight-major, ev/od accumulate into one bank ----
            h_ps = [
                hps.tile([2 * DP, 512], F32, tag="hp", name=f"hps{w}")
                for w in range(2)
            ]
            for half in range(2):
                for w in range(2):
                    nc.tensor.matmul(
                        h_ps[w][:],
                        wsil_at(l, half),
                        st.e[:, half * NP_ + w * 512 : half * NP_ + (w + 1) * 512],
                        start=(half == 0),
                        stop=(half == 1),
                    )
            h_sb = h_tiles[hctr[0] % NH]
            hctr[0] += 1
            for w in range(2):
                nc.vector.tensor_copy(
                    h_sb[:, HALO + w * 512 : HALO + (w + 1) * 512], h_ps[w][:]
                )

            # ---- FSMN conv: 7 pair-shift passes, weight-major ----
            cp = [
                cps.tile([2 * DP, 512], F32, tag="cp", name=f"cp{w}")
                for w in range(2)
            ]
            for kk in range(NK):
                for w in range(2):
                    nc.tensor.matmul(
                        cp[w][:],
                        wconv_at(l, kk),
                        h_sb[:, w * 512 + kk : w * 512 + kk + 512],
                        start=(kk == 0),
                        stop=(kk == NK - 1),
                    )
            # ---- evacuate conv PSUM (+ residual for l>0), o blocked ----
            o_new = op_.tile([2 * DP, NP_], BF16, name="o_sb")
            for w in range(2):
                ws_ = slice(w * 512, (w + 1) * 512)
                if l == 0:
                    nc.vector.tensor_copy(o_new[:, ws_], cp[w][:])
                else:
                    nc.vector.tensor_tensor(o_new[:, ws_], cp[w][:], st.o[:, ws_], OP.add)
            st.o = o_new

        def stage_final(st):
            st.f = fp.tile([DL, T], BF16, name="f_sb")
            expand(st.f, L - 1, L, st.o)

        def stage_batch_out(b, f_tiles):
            pooled = pp.tile([DL, T], BF16, name="pooled")
            nc.vector.tensor_max(pooled[:], f_tiles[0][:], f_tiles[1][:])
            nc.vector.tensor_max(pooled[:], pooled[:], f_tiles[2][:])
            nc.vector.tensor_max(pooled[:], pooled[:], f_tiles[3][:])
            out_sb = osb.tile([S, T], F32, name="out_sb")
            for w in range(T // 512):
                pd = eps.tile([S, 512], F32, tag="pe", name="pd")
                nc.tensor.matmul(pd[:], wd_sb, pooled[:, w * 512 : (w + 1) * 512])
                nc.scalar.activation(
                    out_sb[:, w * 512 : (w + 1) * 512],
                    pd[:],
                    AF.Identity,
                    bias=bd_sb,
                    scale=1.0,
                )
            nc.sync.dma_start(out=out_d[b], in_=out_sb[:])

        # ---- software-pipelined pairs of sequences; the batch output
        # (pool + decode) is deferred into the next pair's layer stream so
        # the PE never waits on it. The last batch pools incrementally so
        # only its decode remains after the final expand ----
        f_by_batch = {b: [None] * C for b in range(BPC)}
        pending_out = [None]
        last_pool = [None]
        npairs = SEQ // 2
        for pair in range(npairs):
            sA, sB = Seq(2 * pair), Seq(2 * pair + 1)
            for st in (sA, sB):
                stage_load(st, chunks=4 if pair == 0 else 1)
            if pair == 0:
                load_wpk()
            for st in (sA, sB):
                stage_unit0(st)
            for l in range(L):
                for st in (sA, sB):
                    stage_layer(st, l)
                if l == 0 and pending_out[0] is not None:
                    b_out, fs = pending_out[0]
                    stage_batch_out(b_out, fs)
                    pending_out[0] = None
                if l == 1 and pair == npairs - 1:
                    # pre-pool the last batch's first two channels
                    fs = f_by_batch[sA.seq // C]
                    lp = pp.tile([DL, T], BF16, tag="poolL", name="last_pool")
                    nc.vector.tensor_max(lp[:], fs[0][:], fs[1][:])
                    last_pool[0] = lp
            for st in (sA, sB):
                stage_final(st)
                f_by_batch[st.seq // C][st.seq % C] = st.f
                if last_pool[0] is not None:
                    nc.vector.tensor_max(last_pool[0][:], last_pool[0][:], st.f[:])
            b = sA.seq // C
            if sB.seq % C == C - 1 and last_pool[0] is None:
                pending_out[0] = (b, f_by_batch[b])
                f_by_batch[b] = [None] * C
        # last batch: pooled tile is ready; only decode remains
        out_sb = osb.tile([S, T], F32, name="out_sb")
        for w in range(T // 512):
            pd = eps.tile([S, 512], F32, tag="pe", name="pd")
            nc.tensor.matmul(pd[:], wd_sb, last_pool[0][:, w * 512 : (w + 1) * 512])
            nc.scalar.activation(
                out_sb[:, w * 512 : (w + 1) * 512],
                pd[:],
                AF.Identity,
                bias=bd_sb,
                scale=1.0,
            )
        nc.sync.dma_start(out=out_d[BPC - 1], in_=out_sb[:])

    nc.compile()
    return nc


_NC = None


def get_nc():
    global _NC
    if _NC is None:
        _NC = build_nc()
    return _NC


def prep_in_maps(x, We0, be0, Ws0, wl0, wr0, We, be, Ws, wl, wr, We2, be2, Wd, bd):
    import ml_dtypes

    bf16 = ml_dtypes.bfloat16

    # x [B, T, C, F] -> [B, C, F, T] with even|odd time blocks
    xb = np.ascontiguousarray(x.transpose(0, 2, 3, 1), dtype=np.float32)
    xt = np.concatenate([xb[..., 0::2], xb[..., 1::2]], axis=-1).astype(bf16)

    ev = 2 * np.arange(DP)
    od = ev + 1

    # expand weights: o blocked -> duplicated halves for row-tiled pairs
    we_list = [We[0], We[1], We[2], We[3], We2]
    wedup = np.stack([np.concatenate([w, w], axis=0) for w in we_list]).astype(bf16)

    # shrink weights, zero-interleaved columns: e -> h interleaved
    ws_list = [Ws0, Ws[0], Ws[1], Ws[2], Ws[3]]
    wsil = np.zeros((L, 2, DL, DL), np.float32)
    for l in range(L):
        wsil[l, 0][:, ev] = ws_list[l]
        wsil[l, 1][:, od] = ws_list[l]

    biases = np.stack([be0, be[0], be[1], be[2], be[3], be2], axis=1).astype(
        np.float32
    )  # [128, 6]

    wl_full = np.concatenate([wl0[None], wl], axis=0)  # [L, 10, 64]
    wr_full = np.concatenate([wr0[None], wr], axis=0)  # [L, 1, 64]
    taps64 = np.concatenate([wl_full, wr_full], axis=1).copy()  # [L, 11, 64], j = d+9
    taps64[:, LO - 1, :] += 1.0  # conv identity term (o = h + left + right)

    # pair-shift conv weights Wc[l, kk][p_in, p_out]: input rows interleaved
    # (2c = ev, 2c+1 = od), output cols blocked (c = ev, c+64 = od).
    # k = kk - 5: ev_out<-ev_in t_{2k}, ev_out<-od_in t_{2k+1},
    # od_out<-ev_in t_{2k-1}, od_out<-od_in t_{2k}
    Wc = np.zeros((L, NK, 2 * DP, 2 * DP), np.float32)
    cc = np.arange(DP)
    for l in range(L):
        for kk in range(NK):
            k = kk - 5
            d = 2 * k
            if -9 <= d <= 1:
                Wc[l, kk][ev, cc] = taps64[l, d + 9]
                Wc[l, kk][od, cc + DP] = taps64[l, d + 9]
            d = 2 * k + 1
            if -9 <= d <= 1:
                Wc[l, kk][od, cc] = taps64[l, d + 9]
            d = 2 * k - 1
            if -9 <= d <= 1:
                Wc[l, kk][ev, cc + DP] = taps64[l, d + 9]
    wconv = np.ascontiguousarray(
        Wc.transpose(2, 0, 1, 3).reshape(2 * DP, L * NK * 2 * DP)
    ).astype(bf16)

    wpk = np.zeros((DL, WPK_COLS), bf16)
    wpk[0:F, OFF_WE0 : OFF_WE0 + DL] = We0.astype(bf16)
    wpk[:, OFF_WEDUP : OFF_WEDUP + L * DL] = wedup.transpose(1, 0, 2).reshape(
        DL, L * DL
    )
    wpk[:, OFF_WSIL : OFF_WSIL + 2 * L * DL] = (
        wsil.reshape(2 * L, DL, DL).transpose(1, 0, 2).reshape(DL, 2 * L * DL)
    ).astype(bf16)
    wpk[:, OFF_WCONV : OFF_WCONV + L * NK * 2 * DP] = wconv
    wpk[:, OFF_WD : OFF_WD + S] = Wd.astype(bf16)

    wpk32 = np.zeros((DL, 8), np.float32)
    wpk32[:, 0 : L + 1] = biases
    wpk32[0:S, 6] = bd

    shared = dict(
        we0=np.ascontiguousarray(We0.astype(bf16)),
        wpk=np.ascontiguousarray(wpk),
        wpk32=wpk32,
    )
    in_maps = []
    for k in range(NCORES):
        xs = xt[k * BPC : (k + 1) * BPC].reshape(SEQ, F, T)
        m = dict(shared)
        m["xt"] = np.ascontiguousarray(xs)
        in_maps.append(m)
    return in_maps


def postprocess(results):
    full = np.concatenate([r["out"] for r in results], axis=0)  # [B, S, T] ev|od
    res = np.empty((B, T, S), np.float32)
    res[:, 0::2, :] = full[:, :, :NP_].transpose(0, 2, 1)
    res[:, 1::2, :] = full[:, :, NP_:].transpose(0, 2, 1)
    return res


def kernel(**inputs):
    nc = get_nc()
    in_maps = prep_in_maps(**inputs)
    res = run_bass_kernel_spmd(nc, in_maps, core_ids=list(range(NCORES)))
    return postprocess(res.results)


# revision 26
# speedup vs baseline: 1.0058x; 1.0058x over previous
"""Trainium2 Bass kernel for nn_FSMNSeleNetV3 (FSMN stack + channel maxpool + decoder).

Self-contained: hardcodes all shapes from the problem spec and only imports
numpy + the concourse stack from /opt/trn_rl_repo.

Sharding: pure data parallel over batch. Each of the 8 cores processes 4
batches x 4 channels = 16 independent sequences of T=2048 tokens.

Layout: activations use an even/odd time-split layout, all in bf16 (fp32
PSUM accumulation). The 128-dim expand stream e is [128 feat, T] with
columns 0:1024 = even times, 1024:2048 = odd times. The 64-dim FSMN h
stream uses an interleaved pair layout (partition 2c = channel c even
times, 2c+1 = odd times); the conv output o uses a blocked pair layout
(partitions 0:63 = even, 64:127 = odd) so the expand can run as K=64
row-tiled concurrent matmul pairs (duplicated weight halves at PE rows 0
and 64 share the rhs stream).

FSMN conv: in pair layout the 11-tap depthwise conv collapses to 7
pair-shift matmuls per 512-column window. Each pass is a full 128x128 bf16
matmul whose weight is a banded matrix of per-channel 2x2 blocks with
interleaved input rows and blocked output columns (built on the host); all
taps accumulate in fp32 PSUM. The conv identity term (o = h + left +
right) is folded into the k=0 tap weights; the layer residual is folded
into the PSUM evacuation as a DVE tensor_tensor add.

The interleaved h layout keeps every matmul destination at PSUM base
partition 0 (the ISA rejects dst base 64): shrink even/odd are two M=128
matmuls with zero-interleaved weight columns accumulating into the same
bank.

Scheduling: sequences are emitted in software-pipelined pairs (stage-
interleaved) so the PE always has an independent matmul stream to fill
cross-engine latency gaps (keeps the PE HAM clock-gate warm). Matmuls that
share a stationary operand are emitted back-to-back; PSUM evacuations are
single wide-FD instructions. bf16 weights enable fast weight load (FWL).
"""

import sys

sys.path.insert(0, "/opt/trn_rl_repo")
from contextlib import ExitStack

import numpy as np

import concourse.bass as bass  # noqa: F401
import concourse.mybir as mybir
import concourse.tile as tile
from concourse import bacc
from concourse.bass_utils import run_bass_kernel_spmd

F32 = mybir.dt.float32
F32R = mybir.dt.float32r
BF16 = mybir.dt.bfloat16
AF = mybir.ActivationFunctionType
OP = mybir.AluOpType

NCORES = 8
B, T, C, F = 32, 2048, 4, 120
DL, DP, L, LO, RO, S = 128, 64, 5, 10, 1, 5
BPC = B // NCORES  # batches per core
SEQ = BPC * C  # sequences per core
NP_ = T // 2  # pair columns per sequence (1024)
HALO = 5  # left pair halo (k down to -5)
HW_ = HALO + NP_ + 1  # h buffer width: 1030
NK = 7  # pair-shift passes, k = kk - 5 in [-5 .. +1]
NH = 6  # static h buffers

# packed bf16 weight tensor column offsets
OFF_WE0 = 0
OFF_WEDUP = OFF_WE0 + DL
OFF_WSIL = OFF_WEDUP + L * DL
OFF_WCONV = OFF_WSIL + L * 2 * DL
OFF_WD = OFF_WCONV + L * NK * 2 * DP
WPK_COLS = OFF_WD + 8


def build_nc():
    nc = bacc.Bacc("TRN2", target_bir_lowering=False, debug=False, num_devices=NCORES)

    xt_d = nc.dram_tensor("xt", [SEQ, F, T], BF16, kind="ExternalInput")
    we0_d = nc.dram_tensor("we0", [F, DL], BF16, kind="ExternalInput")
    wpk_d = nc.dram_tensor("wpk", [DL, WPK_COLS], BF16, kind="ExternalInput")
    wpk32_d = nc.dram_tensor("wpk32", [DL, 8], F32, kind="ExternalInput")
    out_d = nc.dram_tensor("out", [BPC, S, T], F32, kind="ExternalOutput")

    with tile.TileContext(nc) as tc, ExitStack() as ctx:
        wp = ctx.enter_context(tc.tile_pool(name="weights", bufs=1))
        xp = ctx.enter_context(tc.tile_pool(name="x", bufs=5))
        ep = ctx.enter_context(tc.tile_pool(name="e", bufs=5))
        op_ = ctx.enter_context(tc.tile_pool(name="o", bufs=6))
        fp = ctx.enter_context(tc.tile_pool(name="f", bufs=6))
        pp = ctx.enter_context(tc.tile_pool(name="pooled", bufs=2))
        osb = ctx.enter_context(tc.tile_pool(name="osb", bufs=2))
        eps = ctx.enter_context(tc.tile_pool(name="eps", bufs=4, space="PSUM"))
        hps = ctx.enter_context(tc.tile_pool(name="hps", bufs=2, space="PSUM"))
        cps = ctx.enter_context(tc.tile_pool(name="cps", bufs=2, space="PSUM"))

        # --- weights / constants: tiny early DMAs gate unit-0; the big
        # packed DMA is deferred behind the first x loads ---
        we0_tile = wp.tile([F, DL], BF16)
        nc.sync.dma_start(out=we0_tile[:], in_=we0_d[:])
        wpk32_sb = wp.tile([DL, 8], F32)
        nc.sync.dma_start(out=wpk32_sb[:], in_=wpk32_d[:])
        wpk_sb = wp.tile([DL, WPK_COLS], BF16)

        def load_wpk():
            # expand+shrink weights (cols OFF_WEDUP..OFF_WCONV), then conv
            # weights per layer, then the decoder tail
            nc.sync.dma_start(
                out=wpk_sb[:, OFF_WEDUP:OFF_WCONV], in_=wpk_d[:, OFF_WEDUP:OFF_WCONV]
            )
            for l in range(L):
                c0 = OFF_WCONV + l * NK * 2 * DP
                c1 = OFF_WCONV + (l + 1) * NK * 2 * DP
                nc.sync.dma_start(out=wpk_sb[:, c0:c1], in_=wpk_d[:, c0:c1])
            nc.sync.dma_start(
                out=wpk_sb[:, OFF_WD:WPK_COLS], in_=wpk_d[:, OFF_WD:WPK_COLS]
            )

        we0_sb = we0_tile[:]

        def wedup_at(l, q):
            c = OFF_WEDUP + l * DL
            return wpk_sb[q : q + DP, c : c + DL]

        def wsil_at(l, half):
            c = OFF_WSIL + (l * 2 + half) * DL
            return wpk_sb[:, c : c + DL]

        def wconv_at(l, kk):
            c = OFF_WCONV + (l * NK + kk) * 2 * DP
            return wpk_sb[:, c : c + 2 * DP]

        wd_sb = wpk_sb[:, OFF_WD : OFF_WD + S]
        bias_sb = wpk32_sb
        bd_sb = wpk32_sb[0:S, 6:7]

        # PE warmup: garbage matmuls with no data deps run immediately at
        # kernel start, keeping the HAM clock-gate warm through the initial
        # DMA wait (results are never read)
        warm = eps.tile([DL, 512], F32, tag="pe", name="warm")
        for _ in range(14):
            nc.tensor.matmul(warm[:], wpk_sb[:, 0:DL], wpk_sb[:, 0:512])

        # static h buffers: halo columns zeroed once, data region rewritten
        # per (seq, layer) via the shrink evacuation
        h_tiles = []
        for i in range(NH):
            t = wp.tile([2 * DP, HW_], BF16, tag=f"h{i}", name=f"h{i}")
            nc.gpsimd.memset(t[:, 0:HALO], 0.0)
            nc.gpsimd.memset(t[:, HALO + NP_ : HW_], 0.0)
            h_tiles.append(t)

        class Seq:
            def __init__(self, seq):
                self.seq = seq
                self.e = None
                self.o = None
                self.f = None

        hctr = [0]

        def stage_load(st, chunks=1):
            st.x = xp.tile([F, T], BF16, name="x_sb")
            step = T // chunks
            for i in range(chunks):
                nc.sync.dma_start(
                    out=st.x[:, i * step : (i + 1) * step],
                    in_=xt_d[st.seq][:, i * step : (i + 1) * step],
                )

        def stage_unit0(st):
            st.e = ep.tile([DL, T], BF16, name="e_sb")
            for w in range(4):
                pe = eps.tile([DL, 512], F32, tag="pe", name="pe")
                nc.tensor.matmul(pe[:], we0_sb, st.x[:, w * 512 : (w + 1) * 512])
                nc.scalar.activation(
                    st.e[:, w * 512 : (w + 1) * 512],
                    pe[:],
                    AF.Relu,
                    bias=bias_sb[:, 0:1],
                    scale=1.0,
                )

        def expand(dst_sb, lcol, bias_col, o_prev):
            # o_prev blocked: rows 0:63 = even half, 64:127 = odd half.
            # K=64 row-tiled pairs (weights duplicated at rows 0 and 64)
            # stream concurrently and share the rhs columns.
            for w in range(2):
                ws_ = slice(w * 512, (w + 1) * 512)
                pes = []
                for half in range(2):
                    q = half * DP
                    pe = eps.tile([DL, 512], F32, tag="pe", name="pe")
                    nc.tensor.matmul(
                        pe[:],
                        wedup_at(lcol, q),
                        o_prev[q : q + DP, ws_],
                        tile_position=(q, 0),
                    )
                    pes.append(pe)
                for half in range(2):
                    nc.scalar.activation(
                        dst_sb[:, half * NP_ + w * 512 : half * NP_ + (w + 1) * 512],
                        pes[half][:],
                        AF.Relu,
                        bias=bias_sb[:, bias_col : bias_col + 1],
                        scale=1.0,
                    )

        def stage_layer(st, l):
            if l > 0:
                e_new = ep.tile([DL, T], BF16, name="e_sb")
                expand(e_new, l - 1, l, st.o)
                st.e = e_new

            # ---- shrink l: weight-major, ev/od accumulate into one bank ----
            h_ps = [
                hps.tile([2 * DP, 512], F32, tag="hp", name=f"hps{w}")
                for w in range(2)
            ]
            for half in range(2):
                for w in range(2):
                    nc.tensor.matmul(
                        h_ps[w][:],
                        wsil_at(l, half),
                        st.e[:, half * NP_ + w * 512 : half * NP_ + (w + 1) * 512],
                        start=(half == 0),
                        stop=(half == 1),
                    )
            h_sb = h_tiles[hctr[0] % NH]
            hctr[0] += 1
            for w in range(2):
                nc.vector.tensor_copy(
                    h_sb[:, HALO + w * 512 : HALO + (w + 1) * 512], h_ps[w][:]
                )

            # ---- FSMN conv: 7 pair-shift passes, weight-major ----
            cp = [
                cps.tile([2 * DP, 512], F32, tag="cp", name=f"cp{w}")
                for w in range(2)
            ]
            for kk in range(NK):
                for w in range(2):
                    nc.tensor.matmul(
                        cp[w][:],
                        wconv_at(l, kk),
                        h_sb[:, w * 512 + kk : w * 512 + kk + 512],
                        start=(kk == 0),
                        stop=(kk == NK - 1),
                    )
            # ---- evacuate conv PSUM (+ residual for l>0), o blocked ----
            o_new = op_.tile([2 * DP, NP_], BF16, name="o_sb")
            for w in range(2):
                ws_ = slice(w * 512, (w + 1) * 512)
                if l == 0:
                    nc.vector.tensor_copy(o_new[:, ws_], cp[w][:])
                else:
                    nc.vector.tensor_tensor(o_new[:, ws_], cp[w][:], st.o[:, ws_], OP.add)
            st.o = o_new

        def stage_final(st):
            st.f = fp.tile([DL, T], BF16, name="f_sb")
            expand(st.f, L - 1, L, st.o)

        def stage_batch_out(b, f_tiles):
            pooled = pp.tile([DL, T], BF16, name="pooled")
            nc.vector.tensor_max(pooled[:], f_tiles[0][:], f_tiles[1][:])
            nc.vector.tensor_max(pooled[:], pooled[:], f_tiles[2][:])
            nc.vector.tensor_max(pooled[:], pooled[:], f_tiles[3][:])
            out_sb = osb.tile([S, T], F32, name="out_sb")
            for w in range(T // 512):
                pd = eps.tile([S, 512], F32, tag="pe", name="pd")
                nc.tensor.matmul(pd[:], wd_sb, pooled[:, w * 512 : (w + 1) * 512])
                nc.scalar.activation(
                    out_sb[:, w * 512 : (w + 1) * 512],
                    pd[:],
                    AF.Identity,
                    bias=bd_sb,
                    scale=1.0,
                )
            nc.sync.dma_start(out=out_d[b], in_=out_sb[:])

        # ---- software-pipelined pairs of sequences; the batch output
        # (pool + decode) is deferred into the next pair's layer stream so
        # the PE never waits on it. The last batch pools incrementally so
        # only its decode remains after the final expand ----
        f_by_batch = {b: [None] * C for b in range(BPC)}
        pending_out = [None]
        last_pool = [None]
        npairs = SEQ // 2
        for pair in range(npairs):
            sA, sB = Seq(2 * pair), Seq(2 * pair + 1)
            for st in (sA, sB):
                stage_load(st, chunks=2 if pair == 0 else 1)
            if pair == 0:
                load_wpk()
            for st in (sA, sB):
                stage_unit0(st)
            for l in range(L):
                for st in (sA, sB):
                    stage_layer(st, l)
                if l == 0 and pending_out[0] is not None:
                    b_out, fs = pending_out[0]
                    stage_batch_out(b_out, fs)
                    pending_out[0] = None
                if l == 1 and pair == npairs - 1:
                    # pre-pool the last batch's first two channels
                    fs = f_by_batch[sA.seq // C]
                    lp = pp.tile([DL, T], BF16, tag="poolL", name="last_pool")
                    nc.vector.tensor_max(lp[:], fs[0][:], fs[1][:])
                    last_pool[0] = lp
            for st in (sA, sB):
                stage_final(st)
                f_by_batch[st.seq // C][st.seq % C] = st.f
                if last_pool[0] is not None:
                    nc.vector.tensor_max(last_pool[0][:], last_pool[0][:], st.f[:])
            b = sA.seq // C
            if sB.seq % C == C - 1 and last_pool[0] is None:
                pending_out[0] = (b, f_by_batch[b])
                f_by_batch[b] = [None] * C
        # last batch: pooled tile is ready; only decode remains
        out_sb = osb.tile([S, T], F32, name="out_sb")
        for w in range(T // 512):
            pd = eps.tile([S, 512], F32, tag="pe", name="pd")
            nc.tensor.matmul(pd[:], wd_sb, last_pool[0][:, w * 512 : (w + 1) * 512])
            nc.scalar.activation(
                out_sb[:, w * 512 : (w + 1) * 512],
                pd[:],
                AF.Identity,
                bias=bd_sb,
                scale=1.0,
            )
        nc.sync.dma_start(out=out_d[BPC - 1], in_=out_sb[:])

    nc.compile()
    return nc


_NC = None


def get_nc():
    global _NC
    if _NC is None:
        _NC = build_nc()
    return _NC


def prep_in_maps(x, We0, be0, Ws0, wl0, wr0, We, be, Ws, wl, wr, We2, be2, Wd, bd):
    import ml_dtypes

    bf16 = ml_dtypes.bfloat16

    # x [B, T, C, F] -> [B, C, F, T] with even|odd time blocks
    xb = np.ascontiguousarray(x.transpose(0, 2, 3, 1), dtype=np.float32)
    xt = np.concatenate([xb[..., 0::2], xb[..., 1::2]], axis=-1).astype(bf16)

    ev = 2 * np.arange(DP)
    od = ev + 1

    # expand weights: o blocked -> duplicated halves for row-tiled pairs
    we_list = [We[0], We[1], We[2], We[3], We2]
    wedup = np.stack([np.concatenate([w, w], axis=0) for w in we_list]).astype(bf16)

    # shrink weights, zero-interleaved columns: e -> h interleaved
    ws_list = [Ws0, Ws[0], Ws[1], Ws[2], Ws[3]]
    wsil = np.zeros((L, 2, DL, DL), np.float32)
    for l in range(L):
        wsil[l, 0][:, ev] = ws_list[l]
        wsil[l, 1][:, od] = ws_list[l]

    biases = np.stack([be0, be[0], be[1], be[2], be[3], be2], axis=1).astype(
        np.float32
    )  # [128, 6]

    wl_full = np.concatenate([wl0[None], wl], axis=0)  # [L, 10, 64]
    wr_full = np.concatenate([wr0[None], wr], axis=0)  # [L, 1, 64]
    taps64 = np.concatenate([wl_full, wr_full], axis=1).copy()  # [L, 11, 64], j = d+9
    taps64[:, LO - 1, :] += 1.0  # conv identity term (o = h + left + right)

    # pair-shift conv weights Wc[l, kk][p_in, p_out]: input rows interleaved
    # (2c = ev, 2c+1 = od), output cols blocked (c = ev, c+64 = od).
    # k = kk - 5: ev_out<-ev_in t_{2k}, ev_out<-od_in t_{2k+1},
    # od_out<-ev_in t_{2k-1}, od_out<-od_in t_{2k}
    Wc = np.zeros((L, NK, 2 * DP, 2 * DP), np.float32)
    cc = np.arange(DP)
    for l in range(L):
        for kk in range(NK):
            k = kk - 5
            d = 2 * k
            if -9 <= d <= 1:
                Wc[l, kk][ev, cc] = taps64[l, d + 9]
                Wc[l, kk][od, cc + DP] = taps64[l, d + 9]
            d = 2 * k + 1
            if -9 <= d <= 1:
                Wc[l, kk][od, cc] = taps64[l, d + 9]
            d = 2 * k - 1
            if -9 <= d <= 1:
                Wc[l, kk][ev, cc + DP] = taps64[l, d + 9]
    wconv = np.ascontiguousarray(
        Wc.transpose(2, 0, 1, 3).reshape(2 * DP, L * NK * 2 * DP)
    ).astype(bf16)

    wpk = np.zeros((DL, WPK_COLS), bf16)
    wpk[0:F, OFF_WE0 : OFF_WE0 + DL] = We0.astype(bf16)
    wpk[:, OFF_WEDUP : OFF_WEDUP + L * DL] = wedup.transpose(1, 0, 2).reshape(
        DL, L * DL
    )
    wpk[:, OFF_WSIL : OFF_WSIL + 2 * L * DL] = (
        wsil.reshape(2 * L, DL, DL).transpose(1, 0, 2).reshape(DL, 2 * L * DL)
    ).astype(bf16)
    wpk[:, OFF_WCONV : OFF_WCONV + L * NK * 2 * DP] = wconv
    wpk[:, OFF_WD : OFF_WD + S] = Wd.astype(bf16)

    wpk32 = np.zeros((DL, 8), np.float32)
    wpk32[:, 0 : L + 1] = biases
    wpk32[0:S, 6] = bd

    shared = dict(
        we0=np.ascontiguousarray(We0.astype(bf16)),
        wpk=np.ascontiguousarray(wpk),
        wpk32=wpk32,
    )
    in_maps = []
    for k in range(NCORES):
        xs = xt[k * BPC : (k + 1) * BPC].reshape(SEQ, F, T)
        m = dict(shared)
        m["xt"] = np.ascontiguousarray(xs)
        in_maps.append(m)
    return in_maps


def postprocess(results):
    full = np.concatenate([r["out"] for r in results], axis=0)  # [B, S, T] ev|od
    res = np.empty((B, T, S), np.float32)
    res[:, 0::2, :] = full[:, :, :NP_].transpose(0, 2, 1)
    res[:, 1::2, :] = full[:, :, NP_:].transpose(0, 2, 1)
    return res


def kernel(**inputs):
    nc = get_nc()
    in_maps = prep_in_maps(**inputs)
    res = run_bass_kernel_spmd(nc, in_maps, core_ids=list(range(NCORES)))
    return postprocess(res.results)
